# revision 1
# baseline (speedup 1.0000x reference)
"""Fused linear + cross-entropy loss (BaseChunkLoss) on 8 trn2 NeuronCores.

Strategy (per the sharding hint: token/data parallel):
  - Tokens (N=8192) are sharded 8 ways: each core handles 1024 tokens x the
    full vocab (32000), so every core computes a complete logsumexp for its
    tokens and no cross-device reduction of partials is needed.
  - head_weight streams through each core (262 MB fp32 -> ~360 GB/s DMA,
    overlapped with compute); the 1024-token hidden slice stays resident in
    SBUF.
  - The final tiny reduction - log(s), nll = lse - tgt, weighted mean, and
    the 8-way scalar combine - happens on host, standing in for the
    wrapper's all_reduce of the scalar loss.

Device kernel layout: tokens on PSUM partitions, vocab on the free dim.
  stationary lhsT = hidden^T tile [128 d x 128 tok]
  moving rhs      = weight^T tile [128 d x 500 vocab]
  psum [128 tok x 500 vocab] fp32, accumulated over the D=2048 contraction.
Matmuls run in fp8e4m3 with perf_mode=DoubleRow (2 contraction rows per PE
cell, K=256 per instruction; weights pre-scaled by 64 on-chip for e4m3
range, descaled during the bias add). Per 1500-wide vocab group: DVE does
(psum/64 + bias) in place, DVE extracts the target logit via
(iota == label) * logit with a fused row-sum accumulator, and ACT computes
exp with a fused row-sum accumulator. Set USE_FP8 = False for a bf16
variant (~2.5e-6 loss error instead of ~5e-5, ~1.7x slower).

Host-side input prep is layout-only (transpose/slice/cast of index arrays);
all FLOPs over hidden/weights happen on device inside the measured kernel.
"""
import numpy as np
from contextlib import ExitStack

from concourse import bacc, mybir, tile
from concourse.bass_utils import run_bass_kernel_spmd

F32 = mybir.dt.float32
BF16 = mybir.dt.bfloat16
FP8 = mybir.dt.float8e4
Alu = mybir.AluOpType
Act = mybir.ActivationFunctionType

USE_FP8 = True

N_CORES = 8
N_TOK = 8192
D = 2048
V = 32000
P = 128
KT = D // P            # 16 k-tiles of 128
BANK = 500             # vocab columns per psum bank (<= 512 fp32)
BPG = 3                # banks per vocab group
T = N_TOK // N_CORES   # 1024 tokens per core
T_CONST = T
V_CONST = V
MB = T // P            # 8 token blocks per core

W_SCALE = 64.0         # fp8 weight pre-scale (e4m3 range)
WPAD = 1536            # fp8 W tile inner stride (multiple of 16 for DoubleRow)


def _vocab_groups():
    nbanks = V // BANK
    groups = []
    b = 0
    while b < nbanks:
        nb = min(BPG, nbanks - b)
        groups.append((b * BANK, nb * BANK, nb, b))
        b += nb
    return groups


def _declare_io(nc):
    # h and W arrive pre-transposed from host: h [D, T], W [D, V]
    return (
        nc.declare_dram_parameter("h", [D, T], F32, isOutput=False),
        nc.declare_dram_parameter("W", [D, V], F32, isOutput=False),
        nc.declare_dram_parameter("bias", [V], F32, isOutput=False),
        nc.declare_dram_parameter("iota", [V], F32, isOutput=False),
        nc.declare_dram_parameter("labs", [P, MB], F32, isOutput=False),
        nc.declare_dram_parameter("s_out", [P, MB], F32, isOutput=True),
        nc.declare_dram_parameter("t_out", [P, MB], F32, isOutput=True),
    )


def _postops(nc, pt, nb, nv, bb, ii, labs_t, m, col, junk, ejunk,
             s_cols, t_cols, descale):
    psl = pt[:, 0:nb, 0:BANK]
    bbv = bb[:, 0:nv].rearrange("p (b c) -> p b c", c=BANK)
    iiv = ii[:, 0:nv].rearrange("p (b c) -> p b c", c=BANK)
    if descale:
        nc.vector.scalar_tensor_tensor(
            psl, psl, 1.0 / W_SCALE, bbv, op0=Alu.mult, op1=Alu.add)
    else:
        nc.vector.tensor_tensor(psl, psl, bbv, op=Alu.add)
    jt = junk.tile([P, BPG, BANK], F32, tag="junk")
    nc.vector.scalar_tensor_tensor(
        jt[:, 0:nb, :], iiv, labs_t[:, m:m + 1], psl,
        op0=Alu.is_equal, op1=Alu.mult,
        accum_out=t_cols[:, col:col + 1],
    )
    et = ejunk.tile([P, BPG, BANK], F32, tag="ejunk")
    nc.scalar.activation(
        et[:, 0:nb, :], psl, Act.Exp, accum_out=s_cols[:, col:col + 1])


def _finish(nc, acc, s_cols, t_cols, ng, s_out, t_out):
    s_fin = acc.tile([P, MB], F32, tag="sfin")
    t_fin = acc.tile([P, MB], F32, tag="tfin")
    for m in range(MB):
        nc.vector.tensor_reduce(
            s_fin[:, m:m + 1], s_cols[:, m * ng:(m + 1) * ng],
            axis=mybir.AxisListType.X, op=Alu.add)
        nc.vector.tensor_reduce(
            t_fin[:, m:m + 1], t_cols[:, m * ng:(m + 1) * ng],
            axis=mybir.AxisListType.X, op=Alu.add)
    nc.sync.dma_start(s_out[:], s_fin[:])
    nc.sync.dma_start(t_out[:], t_fin[:])


def _build_bf16():
    groups = _vocab_groups()
    ng = len(groups)
    nc = bacc.Bacc("TRN2", target_bir_lowering=False, debug=False)
    h_d, W_d, bias_d, iota_d, labs_d, s_out, t_out = _declare_io(nc)
    W_r = W_d[:].rearrange("(ko ki) v -> ko ki v", ki=P)   # [KT, 128, V]
    h_r = h_d[:].rearrange("(ko ki) t -> ko ki t", ki=P)   # [KT, 128, T]

    with tile.TileContext(nc) as tc, ExitStack() as ctx:
        hpool = ctx.enter_context(tc.tile_pool(name="hT", bufs=1))
        hstage = ctx.enter_context(tc.tile_pool(name="hstage", bufs=2))
        wpool = ctx.enter_context(tc.tile_pool(name="w", bufs=3))
        wstage = ctx.enter_context(tc.tile_pool(name="wstage", bufs=2))
        bpool = ctx.enter_context(tc.tile_pool(name="bias", bufs=2))
        ipool = ctx.enter_context(tc.tile_pool(name="iota", bufs=2))
        pspool = ctx.enter_context(tc.tile_pool(name="ps", bufs=2, space="PSUM"))
        junk = ctx.enter_context(tc.tile_pool(name="junk", bufs=2))
        ejunk = ctx.enter_context(tc.tile_pool(name="ejunk", bufs=2))
        acc = ctx.enter_context(tc.tile_pool(name="acc", bufs=1))

        labs_t = acc.tile([P, MB], F32, tag="labs")
        nc.sync.dma_start(labs_t[:], labs_d[:])
        s_cols = acc.tile([P, MB * ng], F32, tag="scols")
        t_cols = acc.tile([P, MB * ng], F32, tag="tcols")

        hT = hpool.tile([P, KT, T], BF16, tag="hT")
        for k in range(KT):
            st = hstage.tile([P, T], F32, tag="hstage")
            nc.sync.dma_start(st[:], h_r[k])
            nc.vector.tensor_copy(hT[:, k, :], st[:])

        for voff, nv, nb, col0 in groups:
            wv = wpool.tile([P, KT, BPG * BANK], BF16, tag="w")
            for k in range(KT):
                ws = wstage.tile([P, BPG * BANK], F32, tag="wstage")
                nc.sync.dma_start(ws[:, :nv], W_r[k, :, voff:voff + nv])
                nc.scalar.copy(wv[:, k, :nv], ws[:, :nv])
            bb = bpool.tile([P, BPG * BANK], F32, tag="bias")
            nc.scalar.dma_start(
                bb[:, :nv], bias_d[voff:voff + nv].partition_broadcast(P))
            ii = ipool.tile([P, BPG * BANK], F32, tag="iota")
            nc.scalar.dma_start(
                ii[:, :nv], iota_d[voff:voff + nv].partition_broadcast(P))

            for m in range(MB):
                pt = pspool.tile([P, BPG, 512], F32, tag="ps")
                for k in range(KT):
                    lhsT = hT[:, k, m * P:(m + 1) * P]
                    for bk in range(nb):
                        nc.tensor.matmul(
                            pt[:, bk, 0:BANK], lhsT,
                            wv[:, k, bk * BANK:(bk + 1) * BANK],
                            start=(k == 0), stop=(k == KT - 1),
                        )
                col = m * ng + (col0 // BPG)
                _postops(nc, pt, nb, nv, bb, ii, labs_t, m, col, junk, ejunk,
                         s_cols, t_cols, descale=False)

        _finish(nc, acc, s_cols, t_cols, ng, s_out, t_out)

    nc.compile()
    return nc


def _build_fp8():
    T, V = T_CONST, V_CONST
    """fp8 DoubleRow v5: 4 banks/group; tgt via exact f32 rowdot of gathered
    weight rows (host gathers W[labels]; device does the dot); drain chain is
    one DVE op + one ACT op per psum slot."""
    BPG4 = 4
    GV = BPG4 * BANK            # 2000 vocab per group
    WPAD4 = 2048
    assert V % GV == 0
    MB = T // P
    ng = V // GV
    KP2 = KT // 2

    nc = bacc.Bacc("TRN2", target_bir_lowering=False, debug=False)
    h_d = nc.declare_dram_parameter("h", [D, T], F32, isOutput=False)
    W_d = nc.declare_dram_parameter("W", [D, V], F32, isOutput=False)
    bias_d = nc.declare_dram_parameter("bias", [V], F32, isOutput=False)
    hn_d = nc.declare_dram_parameter("hn", [T, D], F32, isOutput=False)
    wg_d = nc.declare_dram_parameter("wg", [T, D], F32, isOutput=False)
    s_out = nc.declare_dram_parameter("s_out", [P, MB], F32, isOutput=True)
    t_out = nc.declare_dram_parameter("t_out", [P, MB], F32, isOutput=True)

    W_r2 = W_d[:].rearrange("(kp j ki) v -> kp ki j v", ki=P, j=2)
    h_r2 = h_d[:].rearrange("(kp j ki) t -> kp ki j t", ki=P, j=2)

    with tile.TileContext(nc) as tc, ExitStack() as ctx:
        hpool = ctx.enter_context(tc.tile_pool(name="hT", bufs=1))
        hstage = ctx.enter_context(tc.tile_pool(name="hstage", bufs=2))
        wpool = ctx.enter_context(tc.tile_pool(name="w", bufs=2))
        wstage = ctx.enter_context(tc.tile_pool(name="wstage", bufs=2))
        bpool = ctx.enter_context(tc.tile_pool(name="bias", bufs=2))
        gpool = ctx.enter_context(tc.tile_pool(name="gath", bufs=2))
        pspool = ctx.enter_context(tc.tile_pool(name="ps", bufs=2, space="PSUM"))
        ejunk = ctx.enter_context(tc.tile_pool(name="ejunk", bufs=1))
        djunk = ctx.enter_context(tc.tile_pool(name="djunk", bufs=1))
        acc = ctx.enter_context(tc.tile_pool(name="acc", bufs=1))

        s_cols = acc.tile([P, MB * ng], F32, tag="scols")
        t_fin = acc.tile([P, MB], F32, tag="tfin")

        # exact-f32 target logit: per m-block rowdot of hn and gathered rows
        for m in range(MB):
            hg = gpool.tile([P, D], F32, tag="hg")
            nc.scalar.dma_start(hg[:], hn_d[m * P:(m + 1) * P, :])
            wgt = gpool.tile([P, D], F32, tag="wgt")
            nc.scalar.dma_start(wgt[:], wg_d[m * P:(m + 1) * P, :])
            dj = djunk.tile([P, D], F32, tag="djunk")
            nc.vector.tensor_mul(dj[:], hg[:], wgt[:])
            nc.vector.tensor_reduce(
                t_fin[:, m:m + 1], dj[:], axis=mybir.AxisListType.X, op=Alu.add)

        hT = hpool.tile([P, KP2, 2, T], FP8, tag="hT")
        for kp in range(KP2):
            st = hstage.tile([P, 2, T], F32, tag="hstage")
            nc.sync.dma_start(st[:], h_r2[kp])
            nc.vector.tensor_copy(hT[:, kp, :, :], st[:])

        for g in range(ng):
            voff = g * GV
            wv = wpool.tile([P, KP2, 2, WPAD4], FP8, tag="w")
            for kp in range(KP2):
                ws = wstage.tile([P, 2, GV], F32, tag="wstage")
                nc.sync.dma_start(ws[:], W_r2[kp][:, :, voff:voff + GV])
                if kp % 2 == 0:
                    nc.scalar.mul(wv[:, kp, :, 0:GV], ws[:], W_SCALE)
                else:
                    nc.vector.tensor_scalar_mul(wv[:, kp, :, 0:GV], ws[:], W_SCALE)
            bb = bpool.tile([P, GV], F32, tag="bias")
            nc.scalar.dma_start(bb[:], bias_d[voff:voff + GV].partition_broadcast(P))

            for m in range(MB):
                pt = pspool.tile([P, BPG4, 512], F32, tag="ps")
                for kp in range(KP2):
                    lhsT = hT[:, kp, :, m * P:(m + 1) * P]
                    for bk in range(BPG4):
                        nc.tensor.matmul(
                            pt[:, bk, 0:BANK], lhsT,
                            wv[:, kp, :, bk * BANK:(bk + 1) * BANK],
                            start=(kp == 0), stop=(kp == KP2 - 1),
                            perf_mode=mybir.MatmulPerfMode.DoubleRow,
                        )
                col = m * ng + g
                psl = pt[:, 0:BPG4, 0:BANK]
                bbv = bb[:].rearrange("p (b c) -> p b c", c=BANK)
                nc.vector.scalar_tensor_tensor(
                    psl, psl, 1.0 / W_SCALE, bbv, op0=Alu.mult, op1=Alu.add)
                et = ejunk.tile([P, BPG4, BANK], F32, tag="ejunk")
                nc.scalar.activation(
                    et[:], psl, Act.Exp, accum_out=s_cols[:, col:col + 1])

        s_fin = acc.tile([P, MB], F32, tag="sfin")
        for m in range(MB):
            nc.vector.tensor_reduce(
                s_fin[:, m:m + 1], s_cols[:, m * ng:(m + 1) * ng],
                axis=mybir.AxisListType.X, op=Alu.add)
        nc.sync.dma_start(s_out[:], s_fin[:])
        nc.sync.dma_start(t_out[:], t_fin[:])

    nc.compile()
    return nc


_NC_CACHE = {}


def _get_program():
    key = "fp8" if USE_FP8 else "bf16"
    if key not in _NC_CACHE:
        _NC_CACHE[key] = _build_fp8() if USE_FP8 else _build_bf16()
    return _NC_CACHE[key]


def kernel(hidden_states, head_weight, head_bias, loss_weight, labels,
           chunk_size=None, **_unused):
    hidden = np.asarray(hidden_states, dtype=np.float32)
    W = np.asarray(head_weight, dtype=np.float32)
    bias = np.asarray(head_bias, dtype=np.float32)
    lw = np.asarray(loss_weight, dtype=np.float32)
    labels = np.asarray(labels)

    assert hidden.shape == (N_TOK, D) and W.shape == (V, D)

    nc = _get_program()
    Wt = np.ascontiguousarray(W.T)                 # [D, V]
    ht = np.ascontiguousarray(hidden.T)            # [D, N]
    in_maps = []
    if USE_FP8:
        Wg = W[labels.astype(np.int64)]            # gathered rows [N, D]
        for c in range(N_CORES):
            sl = slice(c * T, (c + 1) * T)
            in_maps.append(dict(
                h=np.ascontiguousarray(ht[:, sl]), W=Wt, bias=bias,
                hn=np.ascontiguousarray(hidden[sl]),
                wg=np.ascontiguousarray(Wg[sl])))
    else:
        iota = np.arange(V, dtype=np.float32)
        for c in range(N_CORES):
            sl = slice(c * T, (c + 1) * T)
            labs = labels[sl].reshape(MB, P).T.astype(np.float32).copy()
            in_maps.append(dict(h=np.ascontiguousarray(ht[:, sl]), W=Wt,
                                bias=bias, iota=iota, labs=labs))
    res = run_bass_kernel_spmd(nc, in_maps, list(range(N_CORES)))

    # unshard + host-side scalar combine (the "all_reduce" of the hint)
    s = np.concatenate([r["s_out"].T.reshape(-1) for r in res.results])
    tgt = np.concatenate([r["t_out"].T.reshape(-1) for r in res.results])
    if USE_FP8:
        # device produced the exact f32 dot h.W[label]; add the bias here
        tgt = tgt + bias[labels.astype(np.int64)]
    lse = np.log(s.astype(np.float64))
    nll = lse - tgt.astype(np.float64)
    w64 = lw.astype(np.float64)
    loss = (w64 * nll).sum() / max(w64.sum(), 1.0)
    return np.float32(loss)



# revision 4
# speedup vs baseline: 1.7856x; 1.7856x over previous
"""Fused linear + cross-entropy loss (BaseChunkLoss) on 8 trn2 NeuronCores.

Strategy (per the sharding hint: tensor-parallel over vocab):
  - head_weight is sharded 8 ways over the vocab dim: each core handles the
    FULL 8192 tokens x a 4000-entry vocab slice and produces the partial
    sum_{v in shard} exp(logit[t, v]) for every token.  The cross-device
    logsumexp reduction (sum of the 8 partials, then log) plus the weighted
    mean happen on host, standing in for the wrapper's all_reduce.
  - This puts each core's HBM traffic at ~117 MB (full hidden 67 MB + W
    slice 33 MB + target-row gather 17 MB) -- under the fp8 PE roofline --
    instead of the 290 MB/core a token-sharded design pays to stream the
    whole 262 MB weight through every core.
  - The W slice is cast to fp8 once and stays SBUF-resident; hidden^T
    streams through in 1024-token chunks, cast to fp8 on the fly.

Device kernel layout: tokens on PSUM partitions, vocab on the free dim.
  stationary lhsT = hidden^T tile [128 d x 2 x 128 tok]   (fp8, DoubleRow)
  moving rhs      = weight^T tile [128 d x 2 x 500 vocab]
  psum [128 tok x 500 vocab] fp32, accumulated over D=2048 in 8 matmuls.
Weights are pre-scaled by 64 on-chip for e4m3 range and descaled during the
bias add.  Per 1000-wide vocab group (2 psum banks, 4 groups in flight):
DVE does (psum/64 + bias) in place, ACT computes exp with a fused free-dim
row-sum accumulator into s_cols.  The target logit is computed exactly in
f32 as a DVE rowdot of the core's 1024-token hidden slice against the
host-gathered W[labels] rows; host adds bias[labels].

Host-side input prep is layout-only (transpose/slice/gather of rows); all
FLOPs over hidden/weights happen on device inside the measured kernel.
"""
import numpy as np
from contextlib import ExitStack

from concourse import bacc, mybir, tile
from concourse.bass_utils import run_bass_kernel_spmd

F32 = mybir.dt.float32
FP8 = mybir.dt.float8e4
Alu = mybir.AluOpType
Act = mybir.ActivationFunctionType

N_CORES = 8
N_TOK = 8192
D = 2048
V = 32000
P = 128

VSH = V // N_CORES      # 4000 vocab entries per core
TC = N_TOK // N_CORES   # 1024 tokens per core (for the exact tgt rowdot)
KP2 = D // (2 * P)      # 8 DoubleRow contraction steps of K=256
BANK = 500              # vocab columns per psum bank (<= 512 fp32)
BPG = 2                 # banks per vocab group
GV = BPG * BANK         # 1000 vocab per group
NG = VSH // GV          # 4 groups
CHT = 1024              # tokens per streamed hidden chunk
NCH = N_TOK // CHT      # 8 chunks
MBC = CHT // P          # 8 token blocks per chunk
MBT = N_TOK // P        # 64 token blocks total
HSP = 512               # tokens per hidden DMA piece
DHALF = D // 2          # rowdot split for SBUF economy

W_SCALE = 64.0          # fp8 weight pre-scale (e4m3 range)


def _build():
    nc = bacc.Bacc("TRN2", target_bir_lowering=False, debug=False)
    h_d = nc.declare_dram_parameter("h", [D, N_TOK], F32, isOutput=False)
    W_d = nc.declare_dram_parameter("W", [D, VSH], F32, isOutput=False)
    bias_d = nc.declare_dram_parameter("bias", [VSH], F32, isOutput=False)
    hn_d = nc.declare_dram_parameter("hn", [TC, D], F32, isOutput=False)
    wg_d = nc.declare_dram_parameter("wg", [TC, D], F32, isOutput=False)
    s_out = nc.declare_dram_parameter("s_out", [P, MBT * NG], F32, isOutput=True)
    t_out = nc.declare_dram_parameter("t_out", [P, TC // P * 2], F32, isOutput=True)

    h_r2 = h_d[:].rearrange("(kp j ki) t -> kp ki j t", ki=P, j=2)
    W_r2 = W_d[:].rearrange("(kp j ki) v -> kp ki j v", ki=P, j=2)

    with tile.TileContext(nc) as tc, ExitStack() as ctx:
        wpool = ctx.enter_context(tc.tile_pool(name="w", bufs=1))
        wstage = ctx.enter_context(tc.tile_pool(name="wstage", bufs=2))
        hpool = ctx.enter_context(tc.tile_pool(name="hT", bufs=3))
        hstage = ctx.enter_context(tc.tile_pool(name="hstage", bufs=2))
        bpool = ctx.enter_context(tc.tile_pool(name="bias", bufs=1))
        gpool = ctx.enter_context(tc.tile_pool(name="gath", bufs=2))
        djunk = ctx.enter_context(tc.tile_pool(name="djunk", bufs=1))
        ejunk = ctx.enter_context(tc.tile_pool(name="ejunk", bufs=2))
        pspool = ctx.enter_context(tc.tile_pool(name="ps", bufs=4, space="PSUM"))
        acc = ctx.enter_context(tc.tile_pool(name="acc", bufs=1))

        s_cols = acc.tile([P, MBT * NG], F32, tag="scols")
        t_cols = acc.tile([P, TC // P * 2], F32, tag="tcols")

        bb = bpool.tile([P, VSH], F32, tag="bias")
        nc.sync.dma_start(bb[:], bias_d[:].partition_broadcast(P))

        h_tiles = [None] * NCH

        def stage_h(c):
            hc = hpool.tile([P, KP2, 2, CHT], FP8, tag="hT")
            for kp in range(KP2):
                for s in range(CHT // HSP):
                    t0 = c * CHT + s * HSP
                    st = hstage.tile([P, 2, HSP], F32, tag="hstage")
                    nc.sync.dma_start(st[:], h_r2[kp][:, :, t0:t0 + HSP])
                    nc.gpsimd.tensor_copy(
                        hc[:, kp, :, s * HSP:(s + 1) * HSP], st[:])
            h_tiles[c] = hc

        wv = wpool.tile([P, KP2, 2, VSH], FP8, tag="w")

        def stage_w(g):
            v0 = g * GV
            for kp in range(KP2):
                ws = wstage.tile([P, 2, GV], F32, tag="wstage")
                nc.sync.dma_start(ws[:], W_r2[kp][:, :, v0:v0 + GV])
                nc.gpsimd.tensor_scalar_mul(
                    wv[:, kp, :, v0:v0 + GV], ws[:], W_SCALE)

        def compute(c, mm, g):
            m = c * MBC + mm
            pt = pspool.tile([P, BPG, 512], F32, tag="ps")
            lhsT = h_tiles[c][:, :, :, mm * P:(mm + 1) * P]
            for kp in range(KP2):
                for bk in range(BPG):
                    nc.tensor.matmul(
                        pt[:, bk, 0:BANK], lhsT[:, kp],
                        wv[:, kp, :, g * GV + bk * BANK:g * GV + (bk + 1) * BANK],
                        start=(kp == 0), stop=(kp == KP2 - 1),
                        perf_mode=mybir.MatmulPerfMode.DoubleRow,
                    )
            psl = pt[:, 0:BPG, 0:BANK]
            bbv = bb[:, g * GV:(g + 1) * GV].rearrange("p (b c) -> p b c", c=BANK)
            nc.vector.scalar_tensor_tensor(
                psl, psl, 1.0 / W_SCALE, bbv, op0=Alu.mult, op1=Alu.add)
            et = ejunk.tile([P, BPG, BANK], F32, tag="ejunk")
            col = m * NG + g
            nc.scalar.activation(
                et[:], psl, Act.Exp, accum_out=s_cols[:, col:col + 1])

        def rowdot(r):
            # exact f32 target logit for token block r of this core's slice
            for hh in range(2):
                hg = gpool.tile([P, DHALF], F32, tag="hg")
                nc.sync.dma_start(
                    hg[:], hn_d[r * P:(r + 1) * P, hh * DHALF:(hh + 1) * DHALF])
                wgt = gpool.tile([P, DHALF], F32, tag="wgt")
                nc.sync.dma_start(
                    wgt[:], wg_d[r * P:(r + 1) * P, hh * DHALF:(hh + 1) * DHALF])
                dj = djunk.tile([P, DHALF], F32, tag="djunk")
                nc.vector.tensor_tensor_reduce(
                    dj[:], hg[:], wgt[:], 1.0, 0.0, op0=Alu.mult, op1=Alu.add,
                    accum_out=t_cols[:, r * 2 + hh:r * 2 + hh + 1])

        # -- prologue: W g0, h chunks 0+1, W g1..g3 (DMA queue order) --
        stage_w(0)
        stage_h(0)
        stage_h(1)
        stage_w(1)
        stage_w(2)
        stage_w(3)

        # chunks 0-1 traversed group-major so the PE instruction order
        # matches DMA arrival order (no in-order stalls)
        for g in range(NG):
            for c in (0, 1):
                for mm in range(MBC):
                    compute(c, mm, g)

        # steady state: prefetch chunk c+1, compute chunk c
        stage_h(2)
        for c in range(2, NCH):
            if c + 1 < NCH:
                stage_h(c + 1)
            for mm in range(MBC):
                for g in range(NG):
                    compute(c, mm, g)
            # spread the 8 exact-tgt rowdots over mid-stream chunks
            if 2 <= c <= 5:
                rowdot(2 * (c - 2))
                rowdot(2 * (c - 2) + 1)

        nc.sync.dma_start(s_out[:], s_cols[:])
        nc.sync.dma_start(t_out[:], t_cols[:])

    nc.compile()
    return nc


_NC_CACHE = {}


def _get_program():
    if "v" not in _NC_CACHE:
        _NC_CACHE["v"] = _build()
    return _NC_CACHE["v"]


def kernel(hidden_states, head_weight, head_bias, loss_weight, labels,
           chunk_size=None, **_unused):
    hidden = np.asarray(hidden_states, dtype=np.float32)
    W = np.asarray(head_weight, dtype=np.float32)
    bias = np.asarray(head_bias, dtype=np.float32)
    lw = np.asarray(loss_weight, dtype=np.float32)
    labels = np.asarray(labels).astype(np.int64)

    assert hidden.shape == (N_TOK, D) and W.shape == (V, D)

    nc = _get_program()
    Wt = np.ascontiguousarray(W.T)                 # [D, V]
    ht = np.ascontiguousarray(hidden.T)            # [D, N]
    Wg = W[labels]                                 # gathered rows [N, D]
    in_maps = []
    for c in range(N_CORES):
        vsl = slice(c * VSH, (c + 1) * VSH)
        tsl = slice(c * TC, (c + 1) * TC)
        in_maps.append(dict(
            h=ht,
            W=np.ascontiguousarray(Wt[:, vsl]),
            bias=np.ascontiguousarray(bias[vsl]),
            hn=np.ascontiguousarray(hidden[tsl]),
            wg=np.ascontiguousarray(Wg[tsl]),
        ))
    res = run_bass_kernel_spmd(nc, in_maps, list(range(N_CORES)))

    # unshard + host-side scalar combine (the "all_reduce" of the hint):
    # sum the 8 per-core vocab-shard partials of sum_v exp(logit) per token
    s = np.zeros(N_TOK, dtype=np.float64)
    for r in res.results:
        sc = r["s_out"].astype(np.float64).reshape(P, MBT, NG).sum(axis=2)
        s += sc.T.reshape(N_TOK)
    # exact f32 target dot h . W[label] (+ bias) per token
    tgt = np.concatenate([
        r["t_out"].astype(np.float64).reshape(P, TC // P, 2).sum(axis=2)
        .T.reshape(TC)
        for r in res.results])
    tgt = tgt + bias[labels].astype(np.float64)
    lse = np.log(s)
    nll = lse - tgt
    w64 = lw.astype(np.float64)
    loss = (w64 * nll).sum() / max(w64.sum(), 1.0)
    return np.float32(loss)


# revision 7
# speedup vs baseline: 1.8040x; 1.0103x over previous
"""Fused linear + cross-entropy loss (BaseChunkLoss) on 8 trn2 NeuronCores.

Strategy (per the sharding hint: tensor-parallel over vocab):
  - head_weight is sharded 8 ways over the vocab dim: each core handles the
    FULL 8192 tokens x a 4000-entry vocab slice and produces the partial
    sum_{v in shard} exp(logit[t, v]) for every token.  The cross-device
    logsumexp reduction (sum of the 8 partials, then log) plus the weighted
    mean happen on host, standing in for the wrapper's all_reduce.
  - This puts each core's HBM traffic at ~117 MB (full hidden 67 MB + W
    slice 33 MB + target-row gather 17 MB) -- under the fp8 PE roofline --
    instead of the 290 MB/core a token-sharded design pays to stream the
    whole 262 MB weight through every core.
  - The W slice is cast to fp8 once and stays SBUF-resident; hidden^T
    streams through in 1024-token chunks, cast to fp8 on the fly.

Device kernel layout: tokens on PSUM partitions, vocab on the free dim.
  stationary lhsT = hidden^T tile [128 d x 2 x 128 tok]   (fp8, DoubleRow)
  moving rhs      = weight^T tile [128 d x 2 x 500 vocab]
  psum [128 tok x 500 vocab] fp32, accumulated over D=2048 in 8 matmuls.
Weights are pre-scaled by 64 on-chip for e4m3 range and descaled during the
bias add.  Per 1000-wide vocab group (2 psum banks, 4 groups in flight):
DVE does (psum/64 + bias) in place, ACT computes exp with a fused free-dim
row-sum accumulator into s_cols.  The target logit is computed exactly in
f32 as a DVE rowdot of the core's 1024-token hidden slice against the
host-gathered W[labels] rows; host adds bias[labels].

Host-side input prep is layout-only (transpose/slice/gather of rows); all
FLOPs over hidden/weights happen on device inside the measured kernel.
"""
import numpy as np
from contextlib import ExitStack

from concourse import bacc, mybir, tile
from concourse.bass_utils import run_bass_kernel_spmd

F32 = mybir.dt.float32
FP8 = mybir.dt.float8e4
Alu = mybir.AluOpType
Act = mybir.ActivationFunctionType

N_CORES = 8
N_TOK = 8192
D = 2048
V = 32000
P = 128

VSH = V // N_CORES      # 4000 vocab entries per core
TC = N_TOK // N_CORES   # 1024 tokens per core (for the exact tgt rowdot)
KP2 = D // (2 * P)      # 8 DoubleRow contraction steps of K=256
BANK = 500              # vocab columns per psum bank (<= 512 fp32)
BPG = 2                 # banks per vocab group
GV = BPG * BANK         # 1000 vocab per group
NG = VSH // GV          # 4 groups
CHT = 1024              # tokens per streamed hidden chunk
NCH = N_TOK // CHT      # 8 chunks
MBC = CHT // P          # 8 token blocks per chunk
MBT = N_TOK // P        # 64 token blocks total
HSP = 512               # tokens per hidden DMA piece
DHALF = D // 2          # rowdot split for SBUF economy

W_SCALE = 64.0          # fp8 weight pre-scale (e4m3 range)


def _build():
    nc = bacc.Bacc("TRN2", target_bir_lowering=False, debug=False)
    h_d = nc.declare_dram_parameter("h", [D, N_TOK], F32, isOutput=False)
    W_d = nc.declare_dram_parameter("W", [D, VSH], F32, isOutput=False)
    bias_d = nc.declare_dram_parameter("bias", [VSH], F32, isOutput=False)
    hn_d = nc.declare_dram_parameter("hn", [TC, D], F32, isOutput=False)
    wg_d = nc.declare_dram_parameter("wg", [TC, D], F32, isOutput=False)
    s_out = nc.declare_dram_parameter("s_out", [P, MBT * NG], F32, isOutput=True)
    t_out = nc.declare_dram_parameter("t_out", [P, TC // P * 2], F32, isOutput=True)

    h_r2 = h_d[:].rearrange("(kp j ki) t -> kp ki j t", ki=P, j=2)
    W_r2 = W_d[:].rearrange("(kp j ki) v -> kp ki j v", ki=P, j=2)

    with tile.TileContext(nc) as tc, ExitStack() as ctx:
        wpool = ctx.enter_context(tc.tile_pool(name="w", bufs=1))
        wstage = ctx.enter_context(tc.tile_pool(name="wstage", bufs=2))
        hpool = ctx.enter_context(tc.tile_pool(name="hT", bufs=3))
        hstage = ctx.enter_context(tc.tile_pool(name="hstage", bufs=2))
        bpool = ctx.enter_context(tc.tile_pool(name="bias", bufs=1))
        gpool = ctx.enter_context(tc.tile_pool(name="gath", bufs=2))
        djunk = ctx.enter_context(tc.tile_pool(name="djunk", bufs=1))
        ejunk = ctx.enter_context(tc.tile_pool(name="ejunk", bufs=2))
        pspool = ctx.enter_context(tc.tile_pool(name="ps", bufs=4, space="PSUM"))
        acc = ctx.enter_context(tc.tile_pool(name="acc", bufs=1))

        s_cols = acc.tile([P, MBT * NG], F32, tag="scols")
        t_cols = acc.tile([P, TC // P * 2], F32, tag="tcols")

        bb = bpool.tile([P, VSH], F32, tag="bias")

        def stage_bias(g):
            v0 = g * GV
            nc.sync.dma_start(
                bb[:, v0:v0 + GV], bias_d[v0:v0 + GV].partition_broadcast(P))

        h_tiles = [None] * NCH

        def stage_h(c):
            # piece order s-outer/kp-inner so early token blocks complete
            # (and unblock their matmuls) before the whole chunk lands
            hc = hpool.tile([P, KP2, 2, CHT], FP8, tag="hT")
            for s in range(CHT // HSP):
                for kp in range(KP2):
                    t0 = c * CHT + s * HSP
                    st = hstage.tile([P, 2, HSP], F32, tag="hstage")
                    nc.sync.dma_start(st[:], h_r2[kp][:, :, t0:t0 + HSP])
                    nc.gpsimd.tensor_copy(
                        hc[:, kp, :, s * HSP:(s + 1) * HSP], st[:])
            h_tiles[c] = hc

        wv = wpool.tile([P, KP2, 2, VSH], FP8, tag="w")

        def stage_w(g):
            v0 = g * GV
            for kp in range(KP2):
                ws = wstage.tile([P, 2, GV], F32, tag="wstage")
                nc.sync.dma_start(ws[:], W_r2[kp][:, :, v0:v0 + GV])
                nc.gpsimd.tensor_scalar_mul(
                    wv[:, kp, :, v0:v0 + GV], ws[:], W_SCALE)

        def compute(c, mm, g):
            m = c * MBC + mm
            pt = pspool.tile([P, BPG, 512], F32, tag="ps")
            lhsT = h_tiles[c][:, :, :, mm * P:(mm + 1) * P]
            for kp in range(KP2):
                for bk in range(BPG):
                    nc.tensor.matmul(
                        pt[:, bk, 0:BANK], lhsT[:, kp],
                        wv[:, kp, :, g * GV + bk * BANK:g * GV + (bk + 1) * BANK],
                        start=(kp == 0), stop=(kp == KP2 - 1),
                        perf_mode=mybir.MatmulPerfMode.DoubleRow,
                    )
            psl = pt[:, 0:BPG, 0:BANK]
            bbv = bb[:, g * GV:(g + 1) * GV].rearrange("p (b c) -> p b c", c=BANK)
            nc.vector.scalar_tensor_tensor(
                psl, psl, 1.0 / W_SCALE, bbv, op0=Alu.mult, op1=Alu.add)
            et = ejunk.tile([P, BPG, BANK], F32, tag="ejunk")
            col = m * NG + g
            nc.scalar.activation(
                et[:], psl, Act.Exp, accum_out=s_cols[:, col:col + 1])

        def rowdot(r):
            # exact f32 target logit for token block r of this core's slice
            for hh in range(2):
                hg = gpool.tile([P, DHALF], F32, tag="hg")
                nc.sync.dma_start(
                    hg[:], hn_d[r * P:(r + 1) * P, hh * DHALF:(hh + 1) * DHALF])
                wgt = gpool.tile([P, DHALF], F32, tag="wgt")
                nc.sync.dma_start(
                    wgt[:], wg_d[r * P:(r + 1) * P, hh * DHALF:(hh + 1) * DHALF])
                dj = djunk.tile([P, DHALF], F32, tag="djunk")
                nc.vector.tensor_tensor_reduce(
                    dj[:], hg[:], wgt[:], 1.0, 0.0, op0=Alu.mult, op1=Alu.add,
                    accum_out=t_cols[:, r * 2 + hh:r * 2 + hh + 1])

        # -- prologue: interleave W groups, bias slices and h chunks on the
        # DMA queue; traverse compute in the same order the data arrives so
        # the in-order PE stream never waits on a far-future transfer --
        stage_w(0)
        stage_bias(0)
        stage_h(0)
        stage_h(1)
        stage_w(1)
        stage_bias(1)
        stage_h(2)
        stage_w(2)
        stage_bias(2)
        stage_w(3)
        stage_bias(3)

        half = MBC // 2
        for c, g, lo, hi in (
            (0, 0, 0, half), (0, 0, half, MBC),
            (1, 0, 0, half), (1, 0, half, MBC),
            (0, 1, 0, MBC), (1, 1, 0, MBC),
            (2, 0, 0, half), (2, 0, half, MBC), (2, 1, 0, MBC),
            (0, 2, 0, MBC), (1, 2, 0, MBC), (2, 2, 0, MBC),
            (0, 3, 0, MBC), (1, 3, 0, MBC), (2, 3, 0, MBC),
        ):
            for mm in range(lo, hi):
                compute(c, mm, g)

        # steady state: prefetch chunk c+1, compute chunk c
        stage_h(3)
        for c in range(3, NCH):
            if c + 1 < NCH:
                stage_h(c + 1)
            for mm in range(MBC):
                for g in range(NG):
                    compute(c, mm, g)
            # spread the 8 exact-tgt rowdots over mid-stream chunks
            if 3 <= c <= 6:
                rowdot(2 * (c - 3))
                rowdot(2 * (c - 3) + 1)

        nc.sync.dma_start(s_out[:], s_cols[:])
        nc.sync.dma_start(t_out[:], t_cols[:])

    nc.compile()
    return nc


_NC_CACHE = {}


def _get_program():
    if "v" not in _NC_CACHE:
        _NC_CACHE["v"] = _build()
    return _NC_CACHE["v"]


def kernel(hidden_states, head_weight, head_bias, loss_weight, labels,
           chunk_size=None, **_unused):
    hidden = np.asarray(hidden_states, dtype=np.float32)
    W = np.asarray(head_weight, dtype=np.float32)
    bias = np.asarray(head_bias, dtype=np.float32)
    lw = np.asarray(loss_weight, dtype=np.float32)
    labels = np.asarray(labels).astype(np.int64)

    assert hidden.shape == (N_TOK, D) and W.shape == (V, D)

    nc = _get_program()
    Wt = np.ascontiguousarray(W.T)                 # [D, V]
    ht = np.ascontiguousarray(hidden.T)            # [D, N]
    Wg = W[labels]                                 # gathered rows [N, D]
    in_maps = []
    for c in range(N_CORES):
        vsl = slice(c * VSH, (c + 1) * VSH)
        tsl = slice(c * TC, (c + 1) * TC)
        in_maps.append(dict(
            h=ht,
            W=np.ascontiguousarray(Wt[:, vsl]),
            bias=np.ascontiguousarray(bias[vsl]),
            hn=np.ascontiguousarray(hidden[tsl]),
            wg=np.ascontiguousarray(Wg[tsl]),
        ))
    res = run_bass_kernel_spmd(nc, in_maps, list(range(N_CORES)))

    # unshard + host-side scalar combine (the "all_reduce" of the hint):
    # sum the 8 per-core vocab-shard partials of sum_v exp(logit) per token
    s = np.zeros(N_TOK, dtype=np.float64)
    for r in res.results:
        sc = r["s_out"].astype(np.float64).reshape(P, MBT, NG).sum(axis=2)
        s += sc.T.reshape(N_TOK)
    # exact f32 target dot h . W[label] (+ bias) per token
    tgt = np.concatenate([
        r["t_out"].astype(np.float64).reshape(P, TC // P, 2).sum(axis=2)
        .T.reshape(TC)
        for r in res.results])
    tgt = tgt + bias[labels].astype(np.float64)
    lse = np.log(s)
    nll = lse - tgt
    w64 = lw.astype(np.float64)
    loss = (w64 * nll).sum() / max(w64.sum(), 1.0)
    return np.float32(loss)
